# revision 1
# baseline (speedup 1.0000x reference)
"""EnhancedRGCN (3-layer GAT) Trainium2 kernel, 8-core SPMD.

Sharding: destination nodes across 8 cores. Host builds a static padded-CSR
(dst-degree-sorted windows of 128 nodes) whose columns are grouped into 4
src-core-pair blocks so the edge gather can use the hardware dma_gather
(int16 indices address a 2-shard 25088-row sub-table). Table rows live in a
Shared DRAM tensor, 64 floats per row (256B dma_gather granularity), in
[core][p][w] order. Per layer: node phase computes [h | a_s | a_d] with
block-diagonal PE matmuls, an AllGather exchanges the per-core tables,
then per window chunked dma_gathers (<=1024 indices each, the hardware
descriptor-ring limit) fetch the src rows; logits use a DVE leaky-relu and
Exp on the Scalar engine, with the weighted aggregation on Vector/GPSIMD.
Padding slots point at a pad table row (h=0, a_s=-3000): exp underflows
to 0 and h=0 kills the numerator, so no masking is needed.
"""

import os
import sys

sys.path.insert(0, "/opt/trn_rl_repo")

import numpy as np

from concourse import bass, bacc, mybir, tile
from concourse.bass_utils import run_bass_kernel_spmd
from concourse.masks import make_identity

NC = 8
P = 128
NG = 4              # src-core-pair groups
F32 = mybir.dt.float32
I16 = mybir.dt.int16
ALU = mybir.AluOpType
ACT = mybir.ActivationFunctionType

PAD_AS = -3000.0    # pad-row attention logit source value
POOL_MULT_FRAC = float(os.environ.get("POOL_MULT_FRAC", "0.0"))
NO_PRELU = os.environ.get("NO_PRELU", "1") == "1"
GCH = int(os.environ.get("GCH", "8"))


def _host_prep(x, edge_index):
    N = x.shape[0]
    src = np.asarray(edge_index[0], dtype=np.int64)
    dst = np.asarray(edge_index[1], dtype=np.int64)

    npc = (N + NC - 1) // NC
    NW = (npc + P - 1) // P
    NP = NW * P

    # rank of each node within its core (degree-sorted), and its table row
    # (within shard): row = (rank % P) * NW + rank // P
    rank_all = np.empty(N, dtype=np.int64)
    perms = []
    for c in range(NC):
        lo, hi = c * npc, min((c + 1) * npc, N)
        n_loc = hi - lo
        m = (dst >= lo) & (dst < hi)
        dstl = dst[m] - lo
        gsrc = (src[m] // npc) // (NC // NG)
        prof = np.zeros((n_loc, NG), dtype=np.int64)
        np.add.at(prof, (dstl, gsrc), 1)
        # cluster rows with similar per-pair degree profiles into the same
        # 128-row windows to minimize per-(window, pair) column padding
        order = np.lexsort((prof[:, 3], prof[:, 2], prof[:, 1],
                            prof[:, 0]))[::-1]
        perms.append(order + lo)
        rank_of_local = np.empty(n_loc, dtype=np.int64)
        rank_of_local[order] = np.arange(n_loc)
        rank_all[lo:hi] = rank_of_local

    # per-core edge lists with (window, partition, pair-group, src)
    cores = []
    for c in range(NC):
        lo, hi = c * npc, min((c + 1) * npc, N)
        emask = (dst >= lo) & (dst < hi)
        e_src, e_dst = src[emask], dst[emask]
        e_rank = rank_all[e_dst]
        cores.append(dict(n_loc=hi - lo, perm=perms[c],
                          e_src=e_src, e_rank=e_rank))

    # unified per-(window, group) column widths across all cores
    Wg = np.zeros((NW, NG), dtype=np.int64)
    for c in range(NC):
        cc = cores[c]
        g = (cc["e_src"] // npc) // (NC // NG)
        w = cc["e_rank"] // P
        p = cc["e_rank"] % P
        cnt = np.zeros((NW, P, NG), dtype=np.int64)
        np.add.at(cnt, (w, p, g), 1)
        Wg = np.maximum(Wg, cnt.max(axis=1))
    Wg = np.maximum(Wg, 1)
    return cores, NW, NP, npc, rank_all, Wg


def _build_program(NW, NP, Wg, Hs, slopes, n_loc):
    nc = bacc.Bacc("TRN2", target_bir_lowering=False, debug=False,
                   num_devices=NC)
    TBL = NC * NP
    d_w2 = Wg.sum(axis=1).astype(int)          # window total columns
    starts2 = np.concatenate([[0], np.cumsum(d_w2)]).astype(int)
    S2 = int(d_w2.sum())
    dwmax = int(d_w2.max())

    assert 0 < NP - n_loc < P
    p_pad = P - (NP - n_loc)

    x_sh = nc.dram_tensor("x_sh", [P, NW, 32], F32, kind="ExternalInput")
    # wrapped int16 gather indices, one [128, 8*W] segment per (window, group)
    idx_in = nc.dram_tensor("idx_in", [P, 8 * S2], I16, kind="ExternalInput")
    wb_in = nc.dram_tensor("wb_in", [P, 324], F32, kind="ExternalInput")
    bias_in = nc.dram_tensor("bias_in", [P, 96], F32, kind="ExternalInput")
    out_d = nc.dram_tensor("out_d", [P, NW, 32], F32, kind="ExternalOutput")

    tbl_full = nc.dram_tensor("tbl_full", [TBL + P, 64], F32,
                              addr_space="Shared")
    barrier_in = nc.dram_tensor("barrier_in", [1, 8], F32)
    barrier_out = nc.dram_tensor("barrier_out", [NC, 8], F32,
                                 addr_space="Shared")
    tbl_sh = nc.dram_tensor("tbl_sh", [P, NW, 64], F32)

    frac = POOL_MULT_FRAC

    with tile.TileContext(nc) as tc:
        with (
            tc.tile_pool(name="res", bufs=1) as res,
            tc.tile_pool(name="xTp", bufs=2) as xTp,
            tc.tile_pool(name="ptp", bufs=2, space="PSUM") as ptp,
            tc.tile_pool(name="pvp", bufs=4, space="PSUM") as pvp,
            tc.tile_pool(name="gp", bufs=2) as gp,
            tc.tile_pool(name="ip", bufs=3) as ip,
            tc.tile_pool(name="tp", bufs=3) as tp,
            tc.tile_pool(name="ep", bufs=3) as ep,
            tc.tile_pool(name="tmpp", bufs=2) as tmpp,
        ):
            ident = res.tile([P, P], F32)
            make_identity(nc, ident[:])
            wb_t = res.tile([P, 324], F32)
            nc.sync.dma_start(wb_t[:], wb_in[:])
            bias_t = res.tile([P, 96], F32)
            nc.sync.dma_start(bias_t[:], bias_in[:])

            nv_all = res.tile([P, NW, 34], F32)
            ad_all = res.tile([P, NW, 2], F32)
            agg_all = res.tile([P, NW, 32], F32)
            xact_all = res.tile([P, NW, 32], F32)
            tneg = res.tile([P, NW, 32], F32)
            den_all = res.tile([P, NW, 2], F32)
            r_all = res.tile([P, NW, 2], F32)

            padc = res.tile([P - p_pad, 34], F32)
            nc.vector.memset(padc[:, 0:32], 0.0)
            nc.vector.memset(padc[:, 32:34], PAD_AS)
            nc.sync.dma_start(tbl_sh[p_pad:P, NW - 1, 0:34], padc[:])

            nc.sync.dma_start(xact_all[:],
                              x_sh[:].rearrange("p w f -> p (w f)"))

            for l in range(3):
                H = Hs[l]
                CH = 32 // H
                slope = float(slopes[l])

                # ---- node phase ----
                if l > 0:
                    bslc = bias_t[:, (l - 1) * 32:l * 32]
                    nc.vector.tensor_tensor(
                        out=xact_all[:], in0=agg_all[:],
                        in1=bslc.unsqueeze(1).to_broadcast([P, NW, 32]),
                        op=ALU.add)
                    nc.vector.tensor_scalar_min(tneg[:], xact_all[:], 0.0)
                    nc.scalar.activation(tneg[:], tneg[:], ACT.Exp)
                    nc.vector.tensor_scalar_max(xact_all[:], xact_all[:], 0.0)
                    nc.vector.scalar_tensor_tensor(
                        out=xact_all[:], in0=tneg[:], scalar=-1.0,
                        in1=xact_all[:], op0=ALU.add, op1=ALU.add)
                    nc.vector.tensor_scalar(
                        out=xact_all[:], in0=xact_all[:],
                        scalar1=3.0, scalar2=-3.0,
                        op0=ALU.min, op1=ALU.max)

                for wb in range(0, NW, 3):
                    cc = min(3, NW - wb)
                    pt = ptp.tile([P, P], F32, tag="pt")
                    nc.tensor.transpose(out=pt[0:cc * 32, :],
                                        in_=xact_all[:, wb:wb + cc, :],
                                        identity=ident[:])
                    xT = xTp.tile([P, P], F32, tag="xT")
                    nc.vector.tensor_copy(xT[0:cc * 32, :], pt[0:cc * 32, :])
                    pv = pvp.tile([P, 108], F32, tag="pv")
                    nc.tensor.matmul(pv[:, 0:36 * cc],
                                     lhsT=xT[0:32 * cc, :],
                                     rhs=wb_t[0:32 * cc,
                                              108 * l:108 * l + 36 * cc],
                                     start=True, stop=True)
                    for wl in range(cc):
                        w = wb + wl
                        nc.vector.tensor_copy(nv_all[:, w, :],
                                              pv[:, 36 * wl:36 * wl + 34])
                        nc.scalar.copy(ad_all[:, w, 0:H],
                                       pv[:, 36 * wl + 32 + H:
                                          36 * wl + 32 + 2 * H])

                # ---- exchange: local table store + AllGather ----
                nc.sync.dma_start(tbl_sh[:, 0:NW - 1, 0:34],
                                  nv_all[:, 0:NW - 1, :])
                nc.sync.dma_start(tbl_sh[0:p_pad, NW - 1, 0:34],
                                  nv_all[0:p_pad, NW - 1, :])
                nc.gpsimd.collective_compute(
                    "AllGather", ALU.bypass,
                    replica_groups=[list(range(NC))],
                    ins=[tbl_sh.ap().opt()],
                    outs=[tbl_full[0:TBL, :].opt()],
                )

                # ---- edge phase ----
                for w in range(NW):
                    dw = int(d_w2[w])
                    s0 = int(starts2[w])
                    G = gp.tile([P, dwmax, 64], F32, tag="G")
                    it = ip.tile([P, 8 * dwmax], I16, tag="it")
                    nc.sync.dma_start(it[:, 0:8 * dw],
                                      idx_in[:, 8 * s0:8 * (s0 + dw)])
                    co = 0
                    for g in range(NG):
                        Wc = int(Wg[w][g])
                        # chunk to <= GCH columns per gather (descriptor ring)
                        for o in range(0, Wc, GCH):
                            wc = min(GCH, Wc - o)
                            nc.gpsimd.dma_gather(
                                out_ap=G[:, co + o:co + o + wc, :],
                                in_ap=tbl_full[g * 2 * NP:
                                               (g + 1) * 2 * NP, :],
                                idxs_ap=it[:, 8 * (co + o):
                                           8 * (co + o + wc)],
                                num_idxs=128 * wc, num_idxs_reg=128 * wc,
                                elem_size=64)
                        co += Wc
                    gsl = G[:, 0:dw, :]
                    t = tp.tile([P, dwmax, 2], F32, tag="t")
                    e = ep.tile([P, dwmax, 2], F32, tag="e")
                    if NO_PRELU:
                        nc.vector.tensor_tensor(
                            out=t[:, 0:dw, 0:H], in0=gsl[:, :, 32:32 + H],
                            in1=ad_all[:, w, 0:H].unsqueeze(1)
                                .to_broadcast([P, dw, H]),
                            op=ALU.add)
                        nc.vector.scalar_tensor_tensor(
                            out=t[:, 0:dw, 0:H], in0=t[:, 0:dw, 0:H],
                            scalar=slope, in1=t[:, 0:dw, 0:H],
                            op0=ALU.mult, op1=ALU.max)
                    else:
                        for h in range(H):
                            nc.scalar.activation(
                                t[:, 0:dw, h], gsl[:, :, 32 + h], ACT.Prelu,
                                bias=ad_all[:, w, h:h + 1], alpha=slope)
                    nc.scalar.activation(e[:, 0:dw, 0:H], t[:, 0:dw, 0:H],
                                         ACT.Exp)
                    nc.vector.tensor_reduce(
                        den_all[:, w, 0:H],
                        e[:, 0:dw, 0:H].transpose([0, 2, 1]),
                        mybir.AxisListType.X, ALU.add)
                    tmp = tmpp.tile([P, dwmax, 32], F32, tag="tmp")
                    use_pool = (int((w + 1) * frac) - int(w * frac)) > 0
                    eng = nc.gpsimd if use_pool else nc.vector
                    for h in range(H):
                        eng.tensor_tensor(
                            out=tmp[:, 0:dw, h * CH:(h + 1) * CH],
                            in0=gsl[:, :, h * CH:(h + 1) * CH],
                            in1=e[:, 0:dw, h].unsqueeze(2)
                                .to_broadcast([P, dw, CH]),
                            op=ALU.mult)
                    nc.vector.tensor_reduce(
                        agg_all[:, w, :],
                        tmp[:, 0:dw, :].transpose([0, 2, 1]),
                        mybir.AxisListType.X, ALU.add)

                # ---- softmax normalization (batched) ----
                nc.vector.tensor_scalar_add(den_all[:, :, 0:H],
                                            den_all[:, :, 0:H], 1e-16)
                nc.vector.reciprocal(r_all[:, :, 0:H], den_all[:, :, 0:H])
                for h in range(H):
                    nc.vector.tensor_tensor(
                        out=agg_all[:, :, h * CH:(h + 1) * CH],
                        in0=agg_all[:, :, h * CH:(h + 1) * CH],
                        in1=r_all[:, :, h].unsqueeze(2)
                            .to_broadcast([P, NW, CH]),
                        op=ALU.mult)

            nc.vector.tensor_tensor(
                out=xact_all[:], in0=agg_all[:],
                in1=bias_t[:, 64:96].unsqueeze(1).to_broadcast([P, NW, 32]),
                op=ALU.add)
            nc.sync.dma_start(out_d[:].rearrange("p w f -> p (w f)"),
                              xact_all[:])

    nc.compile()
    return nc


def kernel(x, edge_index, W1, att_s1, att_d1, b1, ea1,
           W2, att_s2, att_d2, b2, W3, att_s3, att_d3, b3):
    x = np.asarray(x, dtype=np.float32)
    Ws = [np.asarray(W1, np.float32), np.asarray(W2, np.float32),
          np.asarray(W3, np.float32)]
    att_ss = [np.asarray(att_s1, np.float32), np.asarray(att_s2, np.float32),
              np.asarray(att_s3, np.float32)]
    att_ds = [np.asarray(att_d1, np.float32), np.asarray(att_d2, np.float32),
              np.asarray(att_d3, np.float32)]
    bs = [np.asarray(b1, np.float32), np.asarray(b2, np.float32),
          np.asarray(b3, np.float32)]

    s = float(np.tanh(np.asarray(ea1, np.float32))[0])
    if s < 0.1:
        s = 1.0
    c1 = s * 1.05
    Hs = [2, 2, 1]
    slopes = [0.01, 0.2, 0.2]

    N = x.shape[0]
    cores, NW, NP, npc, rank_all, Wg = _host_prep(x, edge_index)
    n_loc = cores[0]["n_loc"]
    assert all(c["n_loc"] == n_loc for c in cores)

    d_w2 = Wg.sum(axis=1).astype(int)
    starts2 = np.concatenate([[0], np.cumsum(d_w2)]).astype(int)
    S2 = int(d_w2.sum())
    goff = np.concatenate(
        [np.zeros((NW, 1), dtype=np.int64), np.cumsum(Wg, axis=1)], axis=1)

    # pad slots gather the even shard's first pad row (local row space)
    r_pad = n_loc
    pad_local = (r_pad % P) * NW + (r_pad // P)

    # block-diagonal fused weights [P, 324] (3 layers x 3-window blocks)
    wb_cat = np.zeros((P, 324), dtype=np.float32)
    for l in range(3):
        W, a_s, a_d = Ws[l], att_ss[l], att_ds[l]
        H = a_s.shape[0]
        CH = a_s.shape[1]
        M = np.zeros((32, 36), dtype=np.float32)
        M[:, :W.shape[0]] = W.T * (c1 if l == 0 else 1.0)
        for h in range(H):
            M[:, 32 + h] = W.T[:, h * CH:(h + 1) * CH] @ a_s[h]
            M[:, 32 + H + h] = W.T[:, h * CH:(h + 1) * CH] @ a_d[h]
        for i in range(3):
            wb_cat[32 * i:32 * (i + 1),
                   108 * l + 36 * i:108 * l + 36 * (i + 1)] = M
    bias_all = np.concatenate([bs[0] * c1, bs[1], bs[2]])
    bias_cat = np.tile(bias_all[None, :], (P, 1)).astype(np.float32)

    in_maps = []
    for c in range(NC):
        cc = cores[c]
        # linear slot index within each (w, g) block: i = col_local*128 + p
        e_src, e_rank = cc["e_src"], cc["e_rank"]
        w = e_rank // P
        p = e_rank % P
        src_core = e_src // npc
        g = src_core // (NC // NG)
        # slot counter within (w, p, g)
        order = np.lexsort((p, g, w))
        wo, po, go_ = w[order], p[order], g[order]
        key = (wo * P + po) * NG + go_
        # rank within same key (consecutive after sort)
        first = np.ones(len(key), dtype=bool)
        first[1:] = key[1:] != key[:-1]
        run_start = np.maximum.accumulate(np.where(first, np.arange(len(key)), 0))
        slot = np.arange(len(key)) - run_start
        # int16 value: (src_core % 2) * NP + table-row within shard
        srank = rank_all[e_src][order]
        val16 = ((src_core[order] % 2) * NP
                 + (srank % P) * NW + (srank // P)).astype(np.int16)
        # fill linear index lists per (w, g)
        lin = np.full((S2, P), pad_local, dtype=np.int16)  # [global col, p]
        gcol = starts2[wo] + goff[wo, go_] + slot
        lin[gcol, po] = val16
        # wrap: per (w, g) segment of n=128*W indices ordered i=(col*128+p):
        # wrapped[j%16 -> partition, j//16 -> free], replicated 8x
        idx16 = np.empty((P, 8 * S2), dtype=np.int16)
        for wdx in range(NW):
            for gg in range(NG):
                c0 = starts2[wdx] + goff[wdx, gg]
                Wc = int(Wg[wdx][gg])
                seg = lin[c0:c0 + Wc, :].reshape(-1)      # i = col*128+p
                wrapped = seg.reshape(-1, 16).T           # [16, n/16]
                idx16[:, 8 * c0:8 * (c0 + Wc)] = np.tile(wrapped, (8, 1))
        xp = x[cc["perm"]]
        xp = np.concatenate(
            [xp, np.zeros((NP - n_loc, 32), np.float32)], axis=0)
        x_pad = np.ascontiguousarray(
            xp.reshape(NW, P, 32).transpose(1, 0, 2))
        in_maps.append({"x_sh": x_pad, "idx_in": idx16,
                        "wb_in": wb_cat, "bias_in": bias_cat})

    nc = _build_program(NW, NP, Wg, Hs, slopes, n_loc)
    global LAST_EXEC_NS, LAST_NC
    LAST_NC = nc
    try:
        from concourse.timeline_sim import TimelineSim
        LAST_EXEC_NS = TimelineSim(nc, no_exec=True).simulate()
    except Exception:
        LAST_EXEC_NS = None
    if os.environ.get("BASS_BUILD_ONLY"):
        return None
    res = run_bass_kernel_spmd(nc, in_maps, list(range(NC)))

    out = np.empty((N, 32), dtype=np.float32)
    for c in range(NC):
        cc = cores[c]
        o = res.results[c]["out_d"]
        o = o.transpose(1, 0, 2).reshape(NP, 32)[:n_loc]
        out[cc["perm"]] = o
    return out



# revision 16
# speedup vs baseline: 1.8482x; 1.8482x over previous
"""EnhancedRGCN (3-layer GAT) Trainium2 kernel, 8-core SPMD.

Sharding: destination nodes across 8 cores (12544 padded rows each, 98
windows of 128). Each layer the full 100352-row node table ([h(32) |
a_s(2) | a_d(2)] as 64-bf16 rows) is addressed by the edge gather in
512B units of FOUR consecutive rows, so one int16 index space (25088
units) covers every source: no src-core grouping, and the
dst-degree-sorted padded CSR has only ~1.5% pad slots. Per window one
dma_gather (<=48 columns = 6144 indices; a descriptor covers 16 indices,
so 385 of the 1024 SWDGE ring slots) fetches the quads; a 1-of-4 select
(ACT copy + 3 DVE copy_predicated on uint32 views with static one-hot
masks) extracts the addressed row. Gathers keep the Pool engine free of
other work so SWDGE descriptor generation pipelines with the DMA
engines; the remaining edge math runs on DVE/ACT below the gather DMA
time.

The layer-0 table needs no exchange at all: x is replicated, so every
core computes the full table with the (otherwise idle) PE engine and
reads its own a_d slice back with one partition-id-offset DMA per chunk.
For layers 1-2 the exchange is software-pipelined: windows are processed
in chunks (16,16,16,16,16,16,2), and after each chunk's edge phase the
next layer's node values for those windows (softmax-normalize + ELU +
block-diagonal PE matmul) are AllGathered into a contiguous chunk slice
of the next layer's (double-buffered) quad table while the remaining
chunks of the current layer are still gathering - only the tiny last
chunk's exchange is exposed. Pad slots point at a pad row (h=0,
a_s=-3000): exp underflows to 0, so no runtime masking is needed.
"""

import os
import sys

sys.path.insert(0, "/opt/trn_rl_repo")

import numpy as np

from concourse import bass, bacc, mybir, tile
from concourse.ap import AP
from concourse.bass_utils import run_bass_kernel_spmd
from concourse.masks import make_identity

NC = 8
P = 128
WBND = [0, 16, 32, 48, 64, 80, 96, 98]   # exchange chunk bounds
K = len(WBND) - 1
F32 = mybir.dt.float32
BF16 = mybir.dt.bfloat16
I16 = mybir.dt.int16
U32 = mybir.dt.uint32
ALU = mybir.AluOpType
ACT = mybir.ActivationFunctionType

PAD_AS = -3000.0    # pad-row attention logit source value
GCH = int(os.environ.get("GCH", "48"))   # gather chunk columns (<=48)


def _host_prep(x, edge_index):
    N = x.shape[0]
    src = np.asarray(edge_index[0], dtype=np.int64)
    dst = np.asarray(edge_index[1], dtype=np.int64)

    npc = (N + NC - 1) // NC
    NW = (npc + P - 1) // P
    NP = NW * P

    # rank of each node within its core (degree-sorted); shard position:
    # window w = rank // P, partition p = rank % P
    rank_all = np.empty(N, dtype=np.int64)
    perms = []
    for c in range(NC):
        lo, hi = c * npc, min((c + 1) * npc, N)
        n_loc = hi - lo
        m = (dst >= lo) & (dst < hi)
        deg = np.bincount(dst[m] - lo, minlength=n_loc)
        order = np.argsort(-deg, kind="stable")
        perms.append(order + lo)
        rank_of_local = np.empty(n_loc, dtype=np.int64)
        rank_of_local[order] = np.arange(n_loc)
        rank_all[lo:hi] = rank_of_local

    cores = []
    for c in range(NC):
        lo, hi = c * npc, min((c + 1) * npc, N)
        emask = (dst >= lo) & (dst < hi)
        e_src, e_dst = src[emask], dst[emask]
        e_rank = rank_all[e_dst]
        cores.append(dict(n_loc=hi - lo, perm=perms[c],
                          e_src=e_src, e_rank=e_rank))

    # unified per-window column widths across all cores
    Wg = np.zeros(NW, dtype=np.int64)
    for c in range(NC):
        cc = cores[c]
        w = cc["e_rank"] // P
        p = cc["e_rank"] % P
        cnt = np.zeros((NW, P), dtype=np.int64)
        np.add.at(cnt, (w, p), 1)
        Wg = np.maximum(Wg, cnt.max(axis=1))
    Wg = np.maximum(Wg, 1)
    return cores, NW, NP, npc, rank_all, Wg


def _build_program(NW, NP, Wg, Hs, slopes, n_loc):
    nc = bacc.Bacc("TRN2", target_bir_lowering=False, debug=False,
                   num_devices=NC)
    TBL = NC * NP
    assert WBND[-1] == NW
    d_w = Wg.astype(int)
    starts = np.concatenate([[0], np.cumsum(d_w)]).astype(int)
    S = int(d_w.sum())
    dwmax = int(d_w.max())
    smax = max(int(starts[WBND[kk + 1]]) - int(starts[WBND[kk]])
               for kk in range(K))

    assert 0 < NP - n_loc < P
    p_pad = P - (NP - n_loc)

    # replicated full input (same array on every core), [p][core][w][f]
    x_full = nc.dram_tensor("x_full", [P, NC, NW, 32], F32,
                            kind="ExternalInput")
    # wrapped int16 gather indices, one [128, 8*W] segment per window
    idx_in = nc.dram_tensor("idx_in", [P, 8 * S], I16, kind="ExternalInput")
    # one-hot quad-select masks (pos==1,2,3) per padded-CSR slot
    m_in = nc.dram_tensor("m_in", [P, 3, S], mybir.dt.uint8,
                          kind="ExternalInput")
    wb_in = nc.dram_tensor("wb_in", [P, 324], F32, kind="ExternalInput")
    bias_in = nc.dram_tensor("bias_in", [P, 96], F32, kind="ExternalInput")
    out_d = nc.dram_tensor("out_d", [P, NW, 32], F32, kind="ExternalOutput")

    # double-buffered quad tables: 512B units of 4 consecutive 64-bf16 rows,
    # chunk k at rows [NC*P*WBND[k], NC*P*WBND[k+1]) in [k][core][p][w']
    # order; global row r = NC*P*w0 + c*P*wk + p*wk + (w-w0)
    tbls = [nc.dram_tensor(f"tbl{i}", [TBL // 4, 256], BF16,
                           addr_space="Shared") for i in range(2)]
    rows0 = tbls[0][:].rearrange("u (k f) -> (u k) f", k=4)
    # per-chunk AllGather staging (this core's [P, wk, 64] shard slice)
    stages = [nc.dram_tensor(f"stage{k}", [P, WBND[k + 1] - WBND[k], 64],
                             BF16) for k in range(K)]

    with tile.TileContext(nc) as tc:
        with (
            tc.tile_pool(name="res", bufs=1) as res,
            tc.tile_pool(name="xfp", bufs=2) as xfp,
            tc.tile_pool(name="nvp", bufs=2) as nvp,
            tc.tile_pool(name="xTp", bufs=2) as xTp,
            tc.tile_pool(name="ptp", bufs=2, space="PSUM") as ptp,
            tc.tile_pool(name="pvp", bufs=4, space="PSUM") as pvp,
            tc.tile_pool(name="gp", bufs=2) as gp,
            tc.tile_pool(name="ip", bufs=3) as ip,
            tc.tile_pool(name="sp", bufs=2) as sp,
            tc.tile_pool(name="tp", bufs=3) as tp,
            tc.tile_pool(name="ep", bufs=3) as ep,
            tc.tile_pool(name="tmpp", bufs=2) as tmpp,
            tc.tile_pool(name="ntp", bufs=2) as ntp,
        ):
            ident = res.tile([P, P], F32)
            make_identity(nc, ident[:])
            wb_t = res.tile([P, 324], F32)
            nc.sync.dma_start(wb_t[:], wb_in[:])
            bias_t = res.tile([P, 96], F32)
            nc.sync.dma_start(bias_t[:], bias_in[:])
            mres = res.tile([P, 3, S], mybir.dt.uint8)
            nc.sync.dma_start(mres[:], m_in[:])

            # pad rows (partitions p_pad.. of the last window): h=0,
            # a_s=PAD_AS
            padc = res.tile([P - p_pad, 64], BF16)
            nc.vector.memset(padc[:], 0.0)
            nc.vector.memset(padc[:, 32:34], PAD_AS)

            nv64 = res.tile([P, NW, 64], BF16)
            nc.vector.memset(nv64[:], 0.0)
            ad_all = res.tile([P, NW, 2], BF16)
            agg_all = res.tile([P, NW, 32], F32)
            xact_all = res.tile([P, NW, 32], F32)
            den_all = res.tile([P, NW, 2], F32)

            def node_block(l, wb, cc, src, dst):
                """[h | a_s | a_d] for windows wb..wb+cc via one PE matmul."""
                pt = ptp.tile([P, P], F32, tag="pt")
                nc.tensor.transpose(out=pt[0:cc * 32, :],
                                    in_=src[:, wb:wb + cc, :],
                                    identity=ident[:])
                xT = xTp.tile([P, P], F32, tag="xT")
                nc.vector.tensor_copy(xT[0:cc * 32, :], pt[0:cc * 32, :])
                pv = pvp.tile([P, 108], F32, tag="pv")
                nc.tensor.matmul(pv[:, 0:36 * cc],
                                 lhsT=xT[0:32 * cc, :],
                                 rhs=wb_t[0:32 * cc,
                                          108 * l:108 * l + 36 * cc],
                                 start=True, stop=True)
                H = Hs[l]
                for wl in range(cc):
                    w = wb + wl
                    nc.scalar.copy(dst[:, w, 0:36],
                                   pv[:, 36 * wl:36 * wl + 36])
                    if dst is nv64:
                        nc.vector.tensor_copy(ad_all[:, w, 0:H],
                                              pv[:, 36 * wl + 32 + H:
                                                 36 * wl + 32 + 2 * H])

            def exchange_chunk(l, k):
                """Ship this core's nv64 chunk into layer-l table chunk k."""
                w0, w1 = WBND[k], WBND[k + 1]
                nc.sync.dma_start(stages[k][:], nv64[:, w0:w1, :])
                if k == K - 1:
                    nc.sync.dma_start(stages[k][p_pad:P, w1 - w0 - 1, :],
                                      padc[:])
                u0, u1 = NC * P * w0 // 4, NC * P * w1 // 4
                nc.gpsimd.collective_compute(
                    "AllGather", ALU.bypass,
                    replica_groups=[list(range(NC))],
                    ins=[stages[k].ap().opt()],
                    outs=[tbls[l % 2][u0:u1, :].opt()],
                )

            # ---- prologue: every core computes the full layer-0 table
            # from the replicated x (PE is idle anyway; no exchange) ----
            for c in range(NC):
                xc = xfp.tile([P, NW, 32], F32, tag="xc")
                nc.sync.dma_start(xc[:], x_full[:, c, :, :])
                nv = nvp.tile([P, NW, 64], BF16, tag="nv")
                nc.vector.memset(nv[:, :, 36:64], 0.0)
                for wb in range(0, NW, 3):
                    node_block(0, wb, min(3, NW - wb), xc, nv)
                for k in range(K):
                    w0, w1 = WBND[k], WBND[k + 1]
                    wk = w1 - w0
                    r0 = NC * P * w0 + c * P * wk
                    nc.sync.dma_start(
                        rows0[r0:r0 + P * wk, :]
                            .rearrange("(p w) f -> p (w f)", p=P),
                        nv[:, w0:w1, :].rearrange("p w f -> p (w f)"))
            # pad rows of core 0's last chunk (referenced by all pad slots):
            # pair-rows (2p, 2p+1) of the 2-window chunk; odd half = w'=1
            rows2 = tbls[0][:].rearrange("u (k f) -> (u k) f", k=2)
            b2 = NC * P * WBND[K - 1] // 2
            nc.sync.dma_start(rows2[b2 + p_pad:b2 + P, 64:128], padc[:])
            # my a_d slice: one partition-id-offset DMA per chunk
            me = nc.sync.partition_id()
            for k in range(K):
                w0, w1 = WBND[k], WBND[k + 1]
                wk = w1 - w0
                src = rows0[NC * P * w0:NC * P * w0 + P * wk, 34:36] \
                    .rearrange("(p w) f -> p w f", p=P)
                src = AP(src.tensor, src.offset + me * (P * wk * 64), src.ap)
                nc.sync.dma_start(ad_all[:, w0:w1, 0:2], src)

            for l in range(3):
                H = Hs[l]
                CH = 32 // H
                slope = float(slopes[l])
                tbl = tbls[l % 2]

                for k in range(K):
                    # ---- edge phase for chunk k's windows ----
                    WK = WBND[k + 1] - WBND[k]
                    for w in range(WBND[k], WBND[k + 1]):
                        dw = int(d_w[w])
                        s0 = int(starts[w])
                        o0 = 0
                        it = ip.tile([P, 8 * dwmax], I16, tag="it")
                        nc.sync.dma_start(it[:, 0:8 * dw],
                                          idx_in[:, 8 * s0:8 * (s0 + dw)])
                        # 36-wide so [.., 0:34] slices stay 3-d strided views
                        sel = sp.tile([P, dwmax, 36], BF16, tag="sel")
                        for o in range(0, dw, GCH):
                            wc = min(GCH, dw - o)
                            G = gp.tile([P, GCH, 256], BF16, tag="G")
                            nc.gpsimd.dma_gather(
                                out_ap=G[:, 0:wc, :],
                                in_ap=tbl[:],
                                idxs_ap=it[:, o0 + 8 * o:
                                           o0 + 8 * (o + wc)],
                                num_idxs=128 * wc, num_idxs_reg=128 * wc,
                                elem_size=256, single_packet=False)
                            # 1-of-4 row select; predicated copies work on
                            # uint32 pairs (the mask is constant across the
                            # row, so pairwise select is equivalent)
                            nc.scalar.copy(sel[:, o:o + wc, 0:34],
                                           G[:, 0:wc, 0:34])
                            for j in range(1, 4):
                                nc.vector.copy_predicated(
                                    sel[:, o:o + wc, 0:34].bitcast(U32),
                                    mres[:, j - 1, s0 + o:s0 + o + wc]
                                        .unsqueeze(2)
                                        .to_broadcast([P, wc, 17]),
                                    G[:, 0:wc, 64 * j:64 * j + 34]
                                        .bitcast(U32))
                        t = tp.tile([P, dwmax, 2], BF16, tag="t")
                        e = ep.tile([P, dwmax, 2], BF16, tag="e")
                        nc.vector.tensor_tensor(
                            out=t[:, 0:dw, 0:H],
                            in0=sel[:, 0:dw, 32:32 + H],
                            in1=ad_all[:, w, 0:H].unsqueeze(1)
                                .to_broadcast([P, dw, H]),
                            op=ALU.add)
                        nc.vector.scalar_tensor_tensor(
                            out=t[:, 0:dw, 0:H], in0=t[:, 0:dw, 0:H],
                            scalar=slope, in1=t[:, 0:dw, 0:H],
                            op0=ALU.mult, op1=ALU.max)
                        nc.scalar.activation(e[:, 0:dw, 0:H],
                                             t[:, 0:dw, 0:H], ACT.Exp)
                        nc.vector.tensor_reduce(
                            den_all[:, w, 0:H],
                            e[:, 0:dw, 0:H].transpose([0, 2, 1]),
                            mybir.AxisListType.X, ALU.add)
                        tmp = tmpp.tile([P, dwmax, 32], BF16, tag="tmp")
                        for h in range(H):
                            nc.vector.tensor_tensor(
                                out=tmp[:, 0:dw, h * CH:(h + 1) * CH],
                                in0=sel[:, 0:dw, h * CH:(h + 1) * CH],
                                in1=e[:, 0:dw, h].unsqueeze(2)
                                    .to_broadcast([P, dw, CH]),
                                op=ALU.mult)
                        nc.vector.tensor_reduce(
                            agg_all[:, w, :],
                            tmp[:, 0:dw, :].transpose([0, 2, 1]),
                            mybir.AxisListType.X, ALU.add)

                    # ---- chunk tail: softmax-normalize; for l<2 compute
                    # and exchange the next layer's node values ----
                    ws = WBND[k]
                    dsl = den_all[:, ws:ws + WK, 0:H]
                    nc.vector.tensor_scalar_add(dsl, dsl, 1e-16)
                    r_t = ntp.tile([P, 16, 2], F32, tag="r")
                    nc.vector.reciprocal(r_t[:, 0:WK, 0:H], dsl)
                    for h in range(H):
                        nc.vector.tensor_tensor(
                            out=agg_all[:, ws:ws + WK,
                                        h * CH:(h + 1) * CH],
                            in0=agg_all[:, ws:ws + WK,
                                        h * CH:(h + 1) * CH],
                            in1=r_t[:, 0:WK, h].unsqueeze(2)
                                .to_broadcast([P, WK, CH]),
                            op=ALU.mult)
                    xsl = xact_all[:, ws:ws + WK, :]
                    bslc = bias_t[:, l * 32:(l + 1) * 32]
                    nc.vector.tensor_tensor(
                        out=xsl, in0=agg_all[:, ws:ws + WK, :],
                        in1=bslc.unsqueeze(1).to_broadcast([P, WK, 32]),
                        op=ALU.add)
                    if l < 2:
                        # ELU (+0.05 detach scale folded for l=0 via wb) and
                        # clip to [-3, 3]
                        tn = ntp.tile([P, 16, 32], F32, tag="tn")
                        nc.vector.tensor_scalar_min(tn[:, 0:WK, :], xsl, 0.0)
                        nc.scalar.activation(tn[:, 0:WK, :], tn[:, 0:WK, :],
                                             ACT.Exp)
                        nc.vector.tensor_scalar_max(xsl, xsl, 0.0)
                        nc.vector.scalar_tensor_tensor(
                            out=xsl, in0=tn[:, 0:WK, :], scalar=-1.0,
                            in1=xsl, op0=ALU.add, op1=ALU.add)
                        nc.vector.tensor_scalar(
                            out=xsl, in0=xsl,
                            scalar1=3.0, scalar2=-3.0,
                            op0=ALU.min, op1=ALU.max)
                        for wb in range(ws, ws + WK, 3):
                            node_block(l + 1, wb, min(3, ws + WK - wb),
                                       xact_all, nv64)
                        exchange_chunk(l + 1, k)

            nc.sync.dma_start(out_d[:].rearrange("p w f -> p (w f)"),
                              xact_all[:])

    nc.compile()
    return nc


def kernel(x, edge_index, W1, att_s1, att_d1, b1, ea1,
           W2, att_s2, att_d2, b2, W3, att_s3, att_d3, b3):
    x = np.asarray(x, dtype=np.float32)
    Ws = [np.asarray(W1, np.float32), np.asarray(W2, np.float32),
          np.asarray(W3, np.float32)]
    att_ss = [np.asarray(att_s1, np.float32), np.asarray(att_s2, np.float32),
              np.asarray(att_s3, np.float32)]
    att_ds = [np.asarray(att_d1, np.float32), np.asarray(att_d2, np.float32),
              np.asarray(att_d3, np.float32)]
    bs = [np.asarray(b1, np.float32), np.asarray(b2, np.float32),
          np.asarray(b3, np.float32)]

    s = float(np.tanh(np.asarray(ea1, np.float32))[0])
    if s < 0.1:
        s = 1.0
    c1 = s * 1.05
    Hs = [2, 2, 1]
    slopes = [0.01, 0.2, 0.2]

    N = x.shape[0]
    cores, NW, NP, npc, rank_all, Wg = _host_prep(x, edge_index)
    n_loc = cores[0]["n_loc"]
    assert all(c["n_loc"] == n_loc for c in cores)

    d_w = Wg.astype(int)
    starts = np.concatenate([[0], np.cumsum(d_w)]).astype(int)
    S = int(d_w.sum())

    # global table row of (core, rank): chunk k of the node's window, then
    # [k][core][p][w - WBND[k]] (AllGather chunk concatenation order)
    wbnd = np.asarray(WBND)
    chunk_of_w = np.searchsorted(wbnd, np.arange(NW), side="right") - 1

    # pad slots gather the quad unit holding core 0's last pad row
    # (partition 127, window NW-1), sub-position 3
    kl = K - 1
    wkl = WBND[kl + 1] - WBND[kl]
    pad_row = (NC * P * WBND[kl] + 0 + (P - 1) * wkl
               + (NW - 1 - WBND[kl]))
    pad_unit = pad_row // 4
    pad_pos = pad_row % 4
    assert pad_pos == 3

    # block-diagonal fused weights [P, 324] (3 layers x 3-window blocks)
    wb_cat = np.zeros((P, 324), dtype=np.float32)
    for l in range(3):
        W, a_s, a_d = Ws[l], att_ss[l], att_ds[l]
        H = a_s.shape[0]
        CH = a_s.shape[1]
        M = np.zeros((32, 36), dtype=np.float32)
        M[:, :W.shape[0]] = W.T * (c1 if l == 0 else 1.0)
        for h in range(H):
            M[:, 32 + h] = W.T[:, h * CH:(h + 1) * CH] @ a_s[h]
            M[:, 32 + H + h] = W.T[:, h * CH:(h + 1) * CH] @ a_d[h]
        for i in range(3):
            wb_cat[32 * i:32 * (i + 1),
                   108 * l + 36 * i:108 * l + 36 * (i + 1)] = M
    bias_all = np.concatenate([bs[0] * c1, bs[1], bs[2]])
    bias_cat = np.tile(bias_all[None, :], (P, 1)).astype(np.float32)

    # replicated [p][core][w][f] layer-0 input (same array on every core)
    x_all = np.empty((P, NC, NW, 32), dtype=np.float32)
    for c in range(NC):
        xp = x[cores[c]["perm"]]
        xp = np.concatenate(
            [xp, np.zeros((NP - n_loc, 32), np.float32)], axis=0)
        x_all[:, c] = xp.reshape(NW, P, 32).transpose(1, 0, 2)

    in_maps = []
    for c in range(NC):
        cc = cores[c]
        e_src, e_rank = cc["e_src"], cc["e_rank"]
        w = e_rank // P
        p = e_rank % P
        # slot counter within (w, p)
        order = np.lexsort((p, w))
        wo, po = w[order], p[order]
        key = wo * P + po
        first = np.ones(len(key), dtype=bool)
        first[1:] = key[1:] != key[:-1]
        run_start = np.maximum.accumulate(
            np.where(first, np.arange(len(key)), 0))
        slot = np.arange(len(key)) - run_start
        # int16 value: global quad unit; pos: row within quad
        srank = rank_all[e_src][order]
        src_core = e_src[order] // npc
        sw = srank // P
        sp_ = srank % P
        skk = chunk_of_w[sw]
        swk = wbnd[skk + 1] - wbnd[skk]
        g_row = (NC * P * wbnd[skk] + src_core * (P * swk) + sp_ * swk
                 + (sw - wbnd[skk]))
        val16 = (g_row // 4).astype(np.int16)
        pos = (g_row % 4).astype(np.int8)
        # fill per-window padded-CSR slots
        lin = np.full((S, P), pad_unit, dtype=np.int16)   # [global col, p]
        pos_arr = np.full((S, P), pad_pos, dtype=np.int8)
        gcol = starts[wo] + slot
        lin[gcol, po] = val16
        pos_arr[gcol, po] = pos
        # wrap per window: segment of n=128*W indices ordered i=(col*128+p)
        # -> [16, n/16] replicated 8x across partitions
        idx16 = np.empty((P, 8 * S), dtype=np.int16)
        for wdx in range(NW):
            c0 = starts[wdx]
            Wc = int(d_w[wdx])
            seg = lin[c0:c0 + Wc, :].reshape(-1)          # i = col*128+p
            wrapped = seg.reshape(-1, 16).T               # [16, n/16]
            idx16[:, 8 * c0:8 * (c0 + Wc)] = np.tile(wrapped, (8, 1))
        # one-hot select masks [P, 3, S]
        m_host = np.zeros((P, 3, S), dtype=np.uint8)
        for j in range(1, 4):
            m_host[:, j - 1, :] = (pos_arr.T == j).astype(np.uint8)
        in_maps.append({"x_full": x_all, "idx_in": idx16, "m_in": m_host,
                        "wb_in": wb_cat, "bias_in": bias_cat})

    nc = _build_program(NW, NP, Wg, Hs, slopes, n_loc)
    global LAST_EXEC_NS, LAST_NC
    LAST_NC = nc
    try:
        from concourse.timeline_sim import TimelineSim
        LAST_EXEC_NS = TimelineSim(nc, no_exec=True).simulate()
    except Exception:
        LAST_EXEC_NS = None
    if os.environ.get("BASS_BUILD_ONLY"):
        return None
    res = run_bass_kernel_spmd(nc, in_maps, list(range(NC)))

    out = np.empty((N, 32), dtype=np.float32)
    for c in range(NC):
        cc = cores[c]
        o = res.results[c]["out_d"]
        o = o.transpose(1, 0, 2).reshape(NP, 32)[:n_loc]
        out[cc["perm"]] = o
    return out


# revision 25
# speedup vs baseline: 1.9361x; 1.0476x over previous
"""EnhancedRGCN (3-layer GAT) Trainium2 kernel, 8-core SPMD.

Sharding: destination nodes across 8 cores (12544 padded rows each, 98
windows of 128). Each layer the full 100352-row node table ([h(32) |
a_s(2) | a_d(2)] as 64-bf16 rows) is addressed by the edge gather in
512B units of FOUR consecutive rows, so one int16 index space (25088
units) covers every source: no src-core grouping, and the
dst-degree-sorted padded CSR has only ~1.5% pad slots. Per window one
dma_gather (<=48 columns = 6144 indices; a descriptor covers 16 indices,
so 385 of the 1024 SWDGE ring slots) fetches the quads; a 1-of-4 select
(ACT copy + 3 DVE copy_predicated on uint32 views with static one-hot
masks) extracts the addressed row. Gathers keep the Pool engine free of
other work so SWDGE descriptor generation pipelines with the DMA
engines; the remaining edge math runs on DVE/ACT below the gather DMA
time.

The layer-0 table needs no exchange at all: x is replicated, so every
core computes the full table with the (otherwise idle) PE engine and
reads its own a_d slice back with one partition-id-offset DMA per chunk.
For layers 1-2 the exchange is software-pipelined: windows are processed
in chunks (16,16,16,16,16,16,2), and after each chunk's edge phase the
next layer's node values for those windows (softmax-normalize + ELU +
block-diagonal PE matmul) are AllGathered into a contiguous chunk slice
of the next layer's (double-buffered) quad table while the remaining
chunks of the current layer are still gathering - only the tiny last
chunk's exchange is exposed. Pad slots point at a pad row (h=0,
a_s=-3000): exp underflows to 0, so no runtime masking is needed.
"""

import os
import sys

sys.path.insert(0, "/opt/trn_rl_repo")

import numpy as np

from concourse import bass, bacc, mybir, tile
from concourse.ap import AP
from concourse.bass_utils import run_bass_kernel_spmd
from concourse.masks import make_identity

NC = 8
P = 128
WBND = [0, 20, 40, 60, 76, 88, 95, 98]   # exchange chunk bounds
K = len(WBND) - 1
F32 = mybir.dt.float32
BF16 = mybir.dt.bfloat16
I16 = mybir.dt.int16
U32 = mybir.dt.uint32
ALU = mybir.AluOpType
ACT = mybir.ActivationFunctionType

PAD_AS = -3000.0    # pad-row attention logit source value
GCH = int(os.environ.get("GCH", "40"))   # gather chunk columns


def _host_prep(x, edge_index):
    N = x.shape[0]
    src = np.asarray(edge_index[0], dtype=np.int64)
    dst = np.asarray(edge_index[1], dtype=np.int64)

    npc = (N + NC - 1) // NC
    NW = (npc + P - 1) // P
    NP = NW * P

    # rank of each node within its core (degree-sorted); shard position:
    # window w = rank // P, partition p = rank % P
    rank_all = np.empty(N, dtype=np.int64)
    perms = []
    for c in range(NC):
        lo, hi = c * npc, min((c + 1) * npc, N)
        n_loc = hi - lo
        m = (dst >= lo) & (dst < hi)
        deg = np.bincount(dst[m] - lo, minlength=n_loc)
        order = np.argsort(-deg, kind="stable")
        perms.append(order + lo)
        rank_of_local = np.empty(n_loc, dtype=np.int64)
        rank_of_local[order] = np.arange(n_loc)
        rank_all[lo:hi] = rank_of_local

    cores = []
    for c in range(NC):
        lo, hi = c * npc, min((c + 1) * npc, N)
        emask = (dst >= lo) & (dst < hi)
        e_src, e_dst = src[emask], dst[emask]
        e_rank = rank_all[e_dst]
        cores.append(dict(n_loc=hi - lo, perm=perms[c],
                          e_src=e_src, e_rank=e_rank))

    # unified per-window column widths across all cores
    Wg = np.zeros(NW, dtype=np.int64)
    for c in range(NC):
        cc = cores[c]
        w = cc["e_rank"] // P
        p = cc["e_rank"] % P
        cnt = np.zeros((NW, P), dtype=np.int64)
        np.add.at(cnt, (w, p), 1)
        Wg = np.maximum(Wg, cnt.max(axis=1))
    Wg = np.maximum(Wg, 1)
    return cores, NW, NP, npc, rank_all, Wg


def _build_program(NW, NP, Wg, Hs, slopes, n_loc):
    nc = bacc.Bacc("TRN2", target_bir_lowering=False, debug=False,
                   num_devices=NC)
    TBL = NC * NP
    assert WBND[-1] == NW
    d_w = Wg.astype(int)
    starts = np.concatenate([[0], np.cumsum(d_w)]).astype(int)
    S = int(d_w.sum())
    dwmax = int(d_w.max())
    smax = max(int(starts[WBND[kk + 1]]) - int(starts[WBND[kk]])
               for kk in range(K))
    WKMAX = max(WBND[kk + 1] - WBND[kk] for kk in range(K))

    assert 0 < NP - n_loc < P
    p_pad = P - (NP - n_loc)

    # replicated full input (same array on every core), [p][core][w][f]
    x_full = nc.dram_tensor("x_full", [P, NC, NW, 32], F32,
                            kind="ExternalInput")
    # wrapped int16 gather indices, one [128, 8*W] segment per window
    idx_in = nc.dram_tensor("idx_in", [P, 8 * S], I16, kind="ExternalInput")
    # one-hot quad-select masks (pos==1,2,3) per padded-CSR slot
    m_in = nc.dram_tensor("m_in", [P, 3, S], mybir.dt.uint8,
                          kind="ExternalInput")
    wb_in = nc.dram_tensor("wb_in", [P, 324], F32, kind="ExternalInput")
    bias_in = nc.dram_tensor("bias_in", [P, 96], F32, kind="ExternalInput")
    out_d = nc.dram_tensor("out_d", [P, NW, 32], F32, kind="ExternalOutput")

    # double-buffered quad tables: 512B units of 4 consecutive 64-bf16 rows,
    # chunk k at rows [NC*P*WBND[k], NC*P*WBND[k+1]) in [k][core][p][w']
    # order; global row r = NC*P*w0 + c*P*wk + p*wk + (w-w0)
    tbls = [nc.dram_tensor(f"tbl{i}", [TBL // 4, 256], BF16,
                           addr_space="Shared") for i in range(2)]
    rows0 = tbls[0][:].rearrange("u (k f) -> (u k) f", k=4)
    # per-chunk AllGather staging (this core's [P, wk, 64] shard slice)
    stages = [nc.dram_tensor(f"stage{k}", [P, WBND[k + 1] - WBND[k], 64],
                             BF16) for k in range(K)]

    with tile.TileContext(nc) as tc:
        with (
            tc.tile_pool(name="res", bufs=1) as res,
            tc.tile_pool(name="xfp", bufs=2) as xfp,
            tc.tile_pool(name="nvp", bufs=2) as nvp,
            tc.tile_pool(name="xTp", bufs=2) as xTp,
            tc.tile_pool(name="ptp", bufs=2, space="PSUM") as ptp,
            tc.tile_pool(name="pvp", bufs=4, space="PSUM") as pvp,
            tc.tile_pool(name="gp", bufs=3) as gp,
            tc.tile_pool(name="ip", bufs=3) as ip,
            tc.tile_pool(name="sp", bufs=2) as sp,
            tc.tile_pool(name="tp", bufs=3) as tp,
            tc.tile_pool(name="ep", bufs=3) as ep,
            tc.tile_pool(name="tmpp", bufs=2) as tmpp,
            tc.tile_pool(name="ntp", bufs=2) as ntp,
        ):
            ident = res.tile([P, P], F32)
            make_identity(nc, ident[:])
            wb_t = res.tile([P, 324], F32)
            nc.sync.dma_start(wb_t[:], wb_in[:])
            bias_t = res.tile([P, 96], F32)
            nc.sync.dma_start(bias_t[:], bias_in[:])
            mres = res.tile([P, 3, S], mybir.dt.uint8)
            nc.sync.dma_start(mres[:], m_in[:])

            # pad rows (partitions p_pad.. of the last window): h=0,
            # a_s=PAD_AS
            padc = res.tile([P - p_pad, 64], BF16)
            nc.vector.memset(padc[:], 0.0)
            nc.vector.memset(padc[:, 32:34], PAD_AS)

            nv64 = res.tile([P, NW, 64], BF16)
            nc.vector.memset(nv64[:], 0.0)
            ad_all = res.tile([P, NW, 2], BF16)
            agg_all = res.tile([P, NW, 32], F32)
            xact_all = res.tile([P, NW, 32], F32)
            den_all = res.tile([P, NW, 2], F32)

            def node_block(l, wb, cc, src, dst):
                """[h | a_s | a_d] for windows wb..wb+cc via one PE matmul."""
                pt = ptp.tile([P, P], F32, tag="pt")
                nc.tensor.transpose(out=pt[0:cc * 32, :],
                                    in_=src[:, wb:wb + cc, :],
                                    identity=ident[:])
                xT = xTp.tile([P, P], F32, tag="xT")
                nc.vector.tensor_copy(xT[0:cc * 32, :], pt[0:cc * 32, :])
                pv = pvp.tile([P, 108], F32, tag="pv")
                nc.tensor.matmul(pv[:, 0:36 * cc],
                                 lhsT=xT[0:32 * cc, :],
                                 rhs=wb_t[0:32 * cc,
                                          108 * l:108 * l + 36 * cc],
                                 start=True, stop=True)
                H = Hs[l]
                pvv = pv[:, 0:36 * cc].rearrange("p (c f) -> p c f", c=cc)
                if dst is nv64:
                    nc.scalar.copy(dst[:, wb:wb + cc, 0:36], pvv)
                    for wl in range(cc):
                        w = wb + wl
                        nc.vector.tensor_copy(ad_all[:, w, 0:H],
                                              pv[:, 36 * wl + 32 + H:
                                                 36 * wl + 32 + 2 * H])
                else:
                    nc.vector.tensor_copy(dst[:, wb:wb + cc, 0:36], pvv)

            def stage_chunk(k):
                """Stage this core's nv64 chunk for its AllGather."""
                w0, w1 = WBND[k], WBND[k + 1]
                nc.sync.dma_start(stages[k][:], nv64[:, w0:w1, :])
                if k == K - 1:
                    nc.sync.dma_start(stages[k][p_pad:P, w1 - w0 - 1, :],
                                      padc[:])

            def ag_chunk(l, k):
                """AllGather staged chunk k into the layer-l table. Emitted
                a few windows into the next chunk so its stage-store wait is
                met by the time it reaches the Pool queue head (it would
                otherwise stall the next chunk's gather descriptor-gen)."""
                w0, w1 = WBND[k], WBND[k + 1]
                u0, u1 = NC * P * w0 // 4, NC * P * w1 // 4
                nc.gpsimd.collective_compute(
                    "AllGather", ALU.bypass,
                    replica_groups=[list(range(NC))],
                    ins=[stages[k].ap().opt()],
                    outs=[tbls[l % 2][u0:u1, :].opt()],
                )

            # ---- prologue: every core computes the full layer-0 table
            # from the replicated x (PE is idle anyway; no exchange) ----
            for c in range(NC):
                xc = xfp.tile([P, NW, 32], F32, tag="xc")
                nc.sync.dma_start(xc[:], x_full[:, c, :, :])
                nv = nvp.tile([P, NW, 64], BF16, tag="nv")
                nc.vector.memset(nv[:, :, 36:64], 0.0)
                for wb in range(0, NW, 3):
                    node_block(0, wb, min(3, NW - wb), xc, nv)
                for k in range(K):
                    w0, w1 = WBND[k], WBND[k + 1]
                    wk = w1 - w0
                    r0 = NC * P * w0 + c * P * wk
                    nc.sync.dma_start(
                        rows0[r0:r0 + P * wk, :]
                            .rearrange("(p w) f -> p (w f)", p=P),
                        nv[:, w0:w1, :].rearrange("p w f -> p (w f)"))
            # pad rows of core 0's last chunk (referenced by all pad slots):
            # pair-rows (2p, 2p+1) of the 2-window chunk; odd half = w'=1
            rows2 = tbls[0][:].rearrange("u (k f) -> (u k) f", k=2)
            b2 = NC * P * WBND[K - 1] // 2
            nc.sync.dma_start(rows2[b2 + p_pad:b2 + P, 64:128], padc[:])
            # my a_d slice: one partition-id-offset DMA per chunk
            me = nc.sync.partition_id()
            for k in range(K):
                w0, w1 = WBND[k], WBND[k + 1]
                wk = w1 - w0
                src = rows0[NC * P * w0:NC * P * w0 + P * wk, 34:36] \
                    .rearrange("(p w) f -> p w f", p=P)
                src = AP(src.tensor, src.offset + me * (P * wk * 64), src.ap)
                nc.sync.dma_start(ad_all[:, w0:w1, 0:2], src)

            for l in range(3):
                H = Hs[l]
                CH = 32 // H
                slope = float(slopes[l])
                tbl = tbls[l % 2]

                pending_ag = None
                for k in range(K):
                    # ---- edge phase for chunk k's windows ----
                    WK = WBND[k + 1] - WBND[k]
                    for w in range(WBND[k], WBND[k + 1]):
                        if pending_ag is not None and w >= WBND[k] + 1:
                            ag_chunk(*pending_ag)
                            pending_ag = None
                        dw = int(d_w[w])
                        s0 = int(starts[w])
                        o0 = 0
                        it = ip.tile([P, 8 * dwmax], I16, tag="it")
                        nc.sync.dma_start(it[:, 0:8 * dw],
                                          idx_in[:, 8 * s0:8 * (s0 + dw)])
                        # 36-wide so [.., 0:34] slices stay 3-d strided views
                        sel = sp.tile([P, dwmax, 36], BF16, tag="sel")
                        for o in range(0, dw, GCH):
                            wc = min(GCH, dw - o)
                            G = gp.tile([P, GCH, 256], BF16, tag="G")
                            nc.gpsimd.dma_gather(
                                out_ap=G[:, 0:wc, :],
                                in_ap=tbl[:],
                                idxs_ap=it[:, o0 + 8 * o:
                                           o0 + 8 * (o + wc)],
                                num_idxs=128 * wc, num_idxs_reg=128 * wc,
                                elem_size=256, single_packet=False)
                            # 1-of-4 row select; predicated copies work on
                            # uint32 pairs (the mask is constant across the
                            # row, so pairwise select is equivalent)
                            nc.scalar.copy(sel[:, o:o + wc, 0:34],
                                           G[:, 0:wc, 0:34])
                            for j in range(1, 4):
                                nc.vector.copy_predicated(
                                    sel[:, o:o + wc, 0:34].bitcast(U32),
                                    mres[:, j - 1, s0 + o:s0 + o + wc]
                                        .unsqueeze(2)
                                        .to_broadcast([P, wc, 17]),
                                    G[:, 0:wc, 64 * j:64 * j + 34]
                                        .bitcast(U32))
                        t = tp.tile([P, dwmax, 2], BF16, tag="t")
                        e = ep.tile([P, dwmax, 2], BF16, tag="e")
                        nc.vector.tensor_tensor(
                            out=t[:, 0:dw, 0:H],
                            in0=sel[:, 0:dw, 32:32 + H],
                            in1=ad_all[:, w, 0:H].unsqueeze(1)
                                .to_broadcast([P, dw, H]),
                            op=ALU.add)
                        nc.vector.scalar_tensor_tensor(
                            out=t[:, 0:dw, 0:H], in0=t[:, 0:dw, 0:H],
                            scalar=slope, in1=t[:, 0:dw, 0:H],
                            op0=ALU.mult, op1=ALU.max)
                        nc.scalar.activation(e[:, 0:dw, 0:H],
                                             t[:, 0:dw, 0:H], ACT.Exp)
                        nc.vector.tensor_reduce(
                            den_all[:, w, 0:H],
                            e[:, 0:dw, 0:H].transpose([0, 2, 1]),
                            mybir.AxisListType.X, ALU.add)
                        tmp = tmpp.tile([P, dwmax, 32], BF16, tag="tmp")
                        for h in range(H):
                            nc.vector.tensor_tensor(
                                out=tmp[:, 0:dw, h * CH:(h + 1) * CH],
                                in0=sel[:, 0:dw, h * CH:(h + 1) * CH],
                                in1=e[:, 0:dw, h].unsqueeze(2)
                                    .to_broadcast([P, dw, CH]),
                                op=ALU.mult)
                        nc.vector.tensor_reduce(
                            agg_all[:, w, :],
                            tmp[:, 0:dw, :].transpose([0, 2, 1]),
                            mybir.AxisListType.X, ALU.add)

                    # ---- chunk tail: softmax-normalize; for l<2 compute
                    # and exchange the next layer's node values ----
                    ws = WBND[k]
                    dsl = den_all[:, ws:ws + WK, 0:H]
                    nc.vector.tensor_scalar_add(dsl, dsl, 1e-16)
                    r_t = ntp.tile([P, WKMAX, 2], F32, tag="r")
                    nc.vector.reciprocal(r_t[:, 0:WK, 0:H], dsl)
                    for h in range(H):
                        nc.vector.tensor_tensor(
                            out=agg_all[:, ws:ws + WK,
                                        h * CH:(h + 1) * CH],
                            in0=agg_all[:, ws:ws + WK,
                                        h * CH:(h + 1) * CH],
                            in1=r_t[:, 0:WK, h].unsqueeze(2)
                                .to_broadcast([P, WK, CH]),
                            op=ALU.mult)
                    xsl = xact_all[:, ws:ws + WK, :]
                    bslc = bias_t[:, l * 32:(l + 1) * 32]
                    nc.vector.tensor_tensor(
                        out=xsl, in0=agg_all[:, ws:ws + WK, :],
                        in1=bslc.unsqueeze(1).to_broadcast([P, WK, 32]),
                        op=ALU.add)
                    if l < 2:
                        # ELU (+0.05 detach scale folded for l=0 via wb) and
                        # clip to [-3, 3]
                        tn = ntp.tile([P, WKMAX, 32], F32, tag="tn")
                        nc.vector.tensor_scalar_min(tn[:, 0:WK, :], xsl, 0.0)
                        nc.scalar.activation(tn[:, 0:WK, :], tn[:, 0:WK, :],
                                             ACT.Exp)
                        nc.vector.tensor_scalar_max(xsl, xsl, 0.0)
                        nc.vector.scalar_tensor_tensor(
                            out=xsl, in0=tn[:, 0:WK, :], scalar=-1.0,
                            in1=xsl, op0=ALU.add, op1=ALU.add)
                        nc.vector.tensor_scalar(
                            out=xsl, in0=xsl,
                            scalar1=3.0, scalar2=-3.0,
                            op0=ALU.min, op1=ALU.max)
                        for wb in range(ws, ws + WK, 3):
                            node_block(l + 1, wb, min(3, ws + WK - wb),
                                       xact_all, nv64)
                        stage_chunk(k)
                        if k == K - 1:
                            if pending_ag is not None:
                                ag_chunk(*pending_ag)
                            ag_chunk(l + 1, k)
                            pending_ag = None
                        else:
                            pending_ag = (l + 1, k)

            nc.sync.dma_start(out_d[:].rearrange("p w f -> p (w f)"),
                              xact_all[:])

    nc.compile()
    return nc


def kernel(x, edge_index, W1, att_s1, att_d1, b1, ea1,
           W2, att_s2, att_d2, b2, W3, att_s3, att_d3, b3):
    x = np.asarray(x, dtype=np.float32)
    Ws = [np.asarray(W1, np.float32), np.asarray(W2, np.float32),
          np.asarray(W3, np.float32)]
    att_ss = [np.asarray(att_s1, np.float32), np.asarray(att_s2, np.float32),
              np.asarray(att_s3, np.float32)]
    att_ds = [np.asarray(att_d1, np.float32), np.asarray(att_d2, np.float32),
              np.asarray(att_d3, np.float32)]
    bs = [np.asarray(b1, np.float32), np.asarray(b2, np.float32),
          np.asarray(b3, np.float32)]

    s = float(np.tanh(np.asarray(ea1, np.float32))[0])
    if s < 0.1:
        s = 1.0
    c1 = s * 1.05
    Hs = [2, 2, 1]
    slopes = [0.01, 0.2, 0.2]

    N = x.shape[0]
    cores, NW, NP, npc, rank_all, Wg = _host_prep(x, edge_index)
    n_loc = cores[0]["n_loc"]
    assert all(c["n_loc"] == n_loc for c in cores)

    d_w = Wg.astype(int)
    starts = np.concatenate([[0], np.cumsum(d_w)]).astype(int)
    S = int(d_w.sum())

    # global table row of (core, rank): chunk k of the node's window, then
    # [k][core][p][w - WBND[k]] (AllGather chunk concatenation order)
    wbnd = np.asarray(WBND)
    chunk_of_w = np.searchsorted(wbnd, np.arange(NW), side="right") - 1

    # pad slots gather the quad unit holding core 0's last pad row
    # (partition 127, window NW-1), sub-position 3
    kl = K - 1
    wkl = WBND[kl + 1] - WBND[kl]
    pad_row = (NC * P * WBND[kl] + 0 + (P - 1) * wkl
               + (NW - 1 - WBND[kl]))
    pad_unit = pad_row // 4
    pad_pos = pad_row % 4
    assert pad_pos == 3

    # block-diagonal fused weights [P, 324] (3 layers x 3-window blocks)
    wb_cat = np.zeros((P, 324), dtype=np.float32)
    for l in range(3):
        W, a_s, a_d = Ws[l], att_ss[l], att_ds[l]
        H = a_s.shape[0]
        CH = a_s.shape[1]
        M = np.zeros((32, 36), dtype=np.float32)
        M[:, :W.shape[0]] = W.T * (c1 if l == 0 else 1.0)
        for h in range(H):
            M[:, 32 + h] = W.T[:, h * CH:(h + 1) * CH] @ a_s[h]
            M[:, 32 + H + h] = W.T[:, h * CH:(h + 1) * CH] @ a_d[h]
        for i in range(3):
            wb_cat[32 * i:32 * (i + 1),
                   108 * l + 36 * i:108 * l + 36 * (i + 1)] = M
    bias_all = np.concatenate([bs[0] * c1, bs[1], bs[2]])
    bias_cat = np.tile(bias_all[None, :], (P, 1)).astype(np.float32)

    # replicated [p][core][w][f] layer-0 input (same array on every core)
    x_all = np.empty((P, NC, NW, 32), dtype=np.float32)
    for c in range(NC):
        xp = x[cores[c]["perm"]]
        xp = np.concatenate(
            [xp, np.zeros((NP - n_loc, 32), np.float32)], axis=0)
        x_all[:, c] = xp.reshape(NW, P, 32).transpose(1, 0, 2)

    in_maps = []
    for c in range(NC):
        cc = cores[c]
        e_src, e_rank = cc["e_src"], cc["e_rank"]
        w = e_rank // P
        p = e_rank % P
        # slot counter within (w, p)
        order = np.lexsort((p, w))
        wo, po = w[order], p[order]
        key = wo * P + po
        first = np.ones(len(key), dtype=bool)
        first[1:] = key[1:] != key[:-1]
        run_start = np.maximum.accumulate(
            np.where(first, np.arange(len(key)), 0))
        slot = np.arange(len(key)) - run_start
        # int16 value: global quad unit; pos: row within quad
        srank = rank_all[e_src][order]
        src_core = e_src[order] // npc
        sw = srank // P
        sp_ = srank % P
        skk = chunk_of_w[sw]
        swk = wbnd[skk + 1] - wbnd[skk]
        g_row = (NC * P * wbnd[skk] + src_core * (P * swk) + sp_ * swk
                 + (sw - wbnd[skk]))
        val16 = (g_row // 4).astype(np.int16)
        pos = (g_row % 4).astype(np.int8)
        # fill per-window padded-CSR slots
        lin = np.full((S, P), pad_unit, dtype=np.int16)   # [global col, p]
        pos_arr = np.full((S, P), pad_pos, dtype=np.int8)
        gcol = starts[wo] + slot
        lin[gcol, po] = val16
        pos_arr[gcol, po] = pos
        # wrap per window: segment of n=128*W indices ordered i=(col*128+p)
        # -> [16, n/16] replicated 8x across partitions
        idx16 = np.empty((P, 8 * S), dtype=np.int16)
        for wdx in range(NW):
            c0 = starts[wdx]
            Wc = int(d_w[wdx])
            seg = lin[c0:c0 + Wc, :].reshape(-1)          # i = col*128+p
            wrapped = seg.reshape(-1, 16).T               # [16, n/16]
            idx16[:, 8 * c0:8 * (c0 + Wc)] = np.tile(wrapped, (8, 1))
        # one-hot select masks [P, 3, S]
        m_host = np.zeros((P, 3, S), dtype=np.uint8)
        for j in range(1, 4):
            m_host[:, j - 1, :] = (pos_arr.T == j).astype(np.uint8)
        in_maps.append({"x_full": x_all, "idx_in": idx16, "m_in": m_host,
                        "wb_in": wb_cat, "bias_in": bias_cat})

    nc = _build_program(NW, NP, Wg, Hs, slopes, n_loc)
    global LAST_EXEC_NS, LAST_NC
    LAST_NC = nc
    try:
        from concourse.timeline_sim import TimelineSim
        LAST_EXEC_NS = TimelineSim(nc, no_exec=True).simulate()
    except Exception:
        LAST_EXEC_NS = None
    if os.environ.get("BASS_BUILD_ONLY"):
        return None
    res = run_bass_kernel_spmd(nc, in_maps, list(range(NC)))

    out = np.empty((N, 32), dtype=np.float32)
    for c in range(NC):
        cc = cores[c]
        o = res.results[c]["out_d"]
        o = o.transpose(1, 0, 2).reshape(NP, 32)[:n_loc]
        out[cc["perm"]] = o
    return out
